# revision 55
# baseline (speedup 1.0000x reference)
"""Trainium2 Bass kernel v4 for the 7-layer binarized CNN (nn_MCNET).

Data parallel over 8 cores (8 images each). Per core:
- L0 (3->4, fp32 input): input split HOST-SIDE into two 11-bit-significand
  fp32 terms (t1 = round11(x), t2 = round11(x - t1), residual <= 2^-22|x|),
  fed to the PE as float32r (TRN2 PE keeps exactly 11 mantissa bits on the
  fp32r moving path, so both terms pass through exactly; verified on HW).
  18 accumulating fp32r matmuls per psum chunk over block-diagonal weights
  (32 row-bands x 3cin on 96 partitions -> 128 out). No on-device
  decomposition.
- Cross-image software pipelining: image j+1's four L0 chunks are emitted
  into image j's early-layer stall points (L1/L2/L3 evac+halo latencies),
  so the PE never drains between layers. PSUM is managed as 4 bank-pairs
  with a round-robin allocator; every multi-chunk tile evacuates per pair
  (two ACT/DVE ops) so pairs free mid-tile.
- Per-psum-chunk ACT Sign -> bf16, DVE 2x2 maxpool writes fp8 directly
  into A1's banded layout.
- L1..L6: fp8e4 DoubleRow matmuls. Activations live in per-layer banded
  buffers A_l: G bands x cin channels on 128 partitions, rows contiguous at
  stride 127 (tap pairs (ki,kj)->(ki+1,kj+1) have ktile stride 128, a legal
  DoubleRow step) -> 6 DoubleRow passes instead of 9; L5 (cin=64) k-splits
  channels across two 4320-byte slabs. Inter-band halos are single
  partition-shifted SBUF DMAs (SP queue); input DMAs ride the idle Pool
  queue; A-layer zero-fill comes from a dram zeros tensor (cheap DMAs).
- Output stored/DMA'd as fp8e4 per row-tile (values in {-1,0,1} exact),
  converted to fp32 on host.
"""
import sys, os, dataclasses
sys.path.insert(0, '/opt/trn_rl_repo')
import numpy as np

CH = [(3, 4), (4, 8), (8, 16), (16, 32), (32, 64), (64, 32), (32, 2)]
G = [32, 32, 16, 8, 4, 4, 4]          # in-bands per layer
WP = 127                              # fp8 row stride (127 % 16 == 15)
SL5 = 4320                            # A5 slab stride (34*127=4318 -> pad to %16)
NIMG = 8
# 5 DoubleRow passes per tap-paired layer: 3 diagonal pairs (ktile stride
# 128 within A), one cross pair (0,2)+(2,2) reading the +2-shifted copy C
# of the activation buffer (stride CBW+256, % 16 == 0), one single (2,0).
PAIRS = [((0, 0), (1, 1)), ((0, 1), (1, 2)), ((1, 0), (2, 1)),
         ((0, 2), (2, 2)), ((2, 0), None)]
NB = [0, 12, 12, 12, 12, 9, 6]        # lhsT block slots per layer (l1..l6)
OFF = {}
_c = 0
for _l in range(1, 7):
    OFF[_l] = _c
    _c += NB[_l] * (256 if _l < 6 else 32)
WF8_COLS = _c
A_ROWS = [0, 6, 10, 18, 34, 0, 34]
A_COLS = [0] + [A_ROWS[l] * WP + 384 for l in range(1, 7)]
A_COLS[5] = 2 * SL5 + 384
# C-copy base (16-aligned) and total buffer width per tap-paired layer
CBW = {l: -(-A_COLS[l] // 16) * 16 for l in (1, 2, 3, 4, 6)}
AW = {l: CBW[l] + A_COLS[l] + 16 for l in (1, 2, 3, 4, 6)}
AW[5] = A_COLS[5]
TSLOT = 5120                          # T cols per slot: 2 terms x 10 rows x 256


def build_program():
    import concourse.bass as bass
    import concourse.mybir as mybir
    dt = mybir.dt
    AF = mybir.ActivationFunctionType
    PM = mybir.MatmulPerfMode
    ALU = mybir.AluOpType

    nc = bass.Bass("TRN2", target_bir_lowering=False)
    x = nc.dram_tensor("x", (NIMG, 2, 3, 258, 256), dt.float32r,
                       kind="ExternalInput")
    w0f = nc.dram_tensor("w0f", (96, 9 * 128), dt.float32r, kind="ExternalInput")
    wf8 = nc.dram_tensor("wf8", (128, WF8_COLS), dt.float8e4, kind="ExternalInput")
    z8 = nc.dram_tensor("z8", (128, 9424), dt.float8e4, kind="ExternalInput")
    y = nc.dram_tensor("y", (NIMG, 2 * 115 * 115), dt.float8e4,
                       kind="ExternalOutput")

    ctxs = []
    def alloc(cm):
        ctxs.append(cm)
        return cm.__enter__()

    W0F = alloc(nc.sbuf_tensor("W0F", [96, 9 * 128], dt.float32r))
    WF8 = alloc(nc.sbuf_tensor("WF8", [128, WF8_COLS], dt.float8e4))
    T = alloc(nc.sbuf_tensor("T", [96, 2 * TSLOT], dt.float32r))
    A = [None] * 7
    for l in range(1, 7):
        A[l] = alloc(nc.sbuf_tensor(f"A{l}", [128, AW[l]], dt.float8e4))
    T0B = alloc(nc.sbuf_tensor("T0B", [128, 2032], dt.bfloat16))
    T1B = alloc(nc.sbuf_tensor("T1B", [128, 4 * 254], dt.bfloat16))
    OUTB = alloc(nc.sbuf_tensor("OUTB", [128, 32 * 115], dt.float8e4))
    PS = alloc(nc.psum_tensor("PS", [128, 4096], dt.float32))
    sem = {n: alloc(nc.semaphore(name=n)) for n in
           ['sdma', 'spe', 'sact', 'sdve', 'sgp', 'sin0', 'sin1', 'swf',
            'sh1', 'sh2', 'sh3', 'sh4', 'sh5', 'sh6', 'sout', 'sout2']}

    def walk(E, me):
        cnt = {'pe': 0, 'act': 0, 'dve': 0}
        last_wait = {}

        def wait(eng, semn, val):
            if val is None or val <= 0:
                return
            k = (eng, semn)
            if last_wait.get(k, -1) >= val:
                return
            last_wait[k] = val
            if eng == me:
                E.wait_ge(sem[semn], val)

        def emit(eng, fn):
            if eng == me:
                return fn()
            return None

        def inc(inst, semn, v):
            if inst is not None:
                inst.then_inc(sem[semn], v)

        # ---- init: weights + A-layer zero-fill on SP queue ----
        # W0F split across SP and DVE queues so the PE can start ~0.9us sooner
        i = emit('sp', lambda: nc.sync.dma_start(W0F[0:96, 0:576], w0f[:, 0:576]))
        inc(i, 'sdma', 16)
        i = emit('act', lambda: nc.scalar.dma_start(W0F[0:96, 576:1152],
                                                    w0f[:, 576:1152]))
        inc(i, 'sdma', 16)
        ms_cnt = 0
        MS_A = {}
        def emit_zero(l):
            nonlocal ms_cnt
            i = emit('sp', lambda l=l: nc.sync.dma_start(
                A[l][0:128, 0:AW[l]], z8[:, 0:AW[l]]))
            ms_cnt += 16
            inc(i, 'sgp', 16)
            MS_A[l] = ms_cnt
        emit_zero(1)
        emit_zero(2)
        i = emit('sp', lambda: nc.sync.dma_start(WF8[0:128, :], wf8[:]))
        inc(i, 'swf', 16)
        for l in (3, 4, 5, 6):
            emit_zero(l)

        # ---- psum pair allocator: 4 pairs of 2 banks (1024 cols each) ----
        pair_free = [None] * 4
        pair_ptr = [0]

        def take_pair():
            p = pair_ptr[0]
            pair_ptr[0] = (p + 1) % 4
            if pair_free[p] is not None:
                wait('pe', pair_free[p][0], pair_free[p][1])
            return p

        hcnt = {l: 0 for l in range(1, 7)}
        out_cnt = [0]
        in_cnt = [0, 0]
        t_free = {}           # img -> spe count when its T slot fully read
        S = [dict() for _ in range(NIMG)]   # per-image state

        def emit_in_dma(j):
            # one DMA per term on the Pool queue (img0: term0 in 2 halves)
            aslot = j % 2
            sname = f'sin{aslot}'
            toff = aslot * TSLOT
            if j >= 2:
                wait('gp', 'spe', t_free[j - 2])
            pieces = []
            if j == 0:
                pieces = [(0, 0, 5), (0, 5, 5), (1, 0, 10)]
            else:
                pieces = [(0, 0, 10), (1, 0, 10)]
            marks = []
            for t, r0, nr in pieces:
                src = dataclasses.replace(
                    x[j, t],
                    offset=x[j, t].offset + r0 * 256,
                    ap=[[2048, 32], [66048, 3], [256, nr], [1, 256]])
                dst0 = toff + t * 2560 + r0 * 256
                i = emit('gp', lambda src=src, dst0=dst0, nr=nr:
                         nc.gpsimd.dma_start(T[0:96, dst0: dst0 + nr * 256], src))
                in_cnt[aslot] += 1
                inc(i, sname, 16)
                marks.append((sname, in_cnt[aslot]))
            S[j]['dma_marks'] = marks

        def l0_mm(jj, c, t, tap, pbase, final):
            aslot = jj % 2
            toff = aslot * TSLOT
            ki, kj = tap // 3, tap % 3
            rbase = toff + t * 2560 + (2 * c + ki) * 256 + kj
            def mk(rbase=rbase, tap=tap, pbase=pbase, t=t):
                lhsT = W0F[0:96, tap * 128: tap * 128 + 128]
                rv = T[0:96, rbase: rbase + 256 + 254]
                rv = dataclasses.replace(rv, ap=[rv.ap[0], [256, 2], [1, 254]])
                ov = PS[0:128, pbase: pbase + 508]
                return nc.tensor.matmul(
                    ov, lhsT, rv, start=(t == 0 and tap == 0),
                    stop=(t == 1 and tap == 8))
            i = emit('pe', mk)
            if final:
                cnt['pe'] += 1
                inc(i, 'spe', 1)

        def emit_hmax(jj, c):
            # hmax (DVE) writes A1 row c: must run after L1 of img jj-1
            st = S[jj]
            wait('dve', 'sdve', st['vmax'][c])   # RAW on T1B
            wait('dve', 'sgp', MS_A[1])
            wait('dve', 'sh1', hcnt[1])
            if jj > 0:
                wait('dve', 'spe', S[jj - 1]['l1_done'])
            def mkh(c=c):
                sv = T1B[0:128, c * 254: c * 254 + 254]
                a = dataclasses.replace(sv, ap=[sv.ap[0], [2, 127]])
                b = dataclasses.replace(sv, offset=sv.offset + 1,
                                        ap=[sv.ap[0], [2, 127]])
                return nc.vector.tensor_max(
                    A[1][0:128, c * WP: c * WP + 127], a, b)
            i = emit('dve', mkh)
            cnt['dve'] += 1
            inc(i, 'sdve', 1)
            a_mark = cnt['dve']
            # C1 copy of the just-written row
            i = emit('dve', lambda c=c: nc.vector.tensor_copy(
                A[1][0:128, CBW[1] + 2 + c * WP: CBW[1] + 2 + c * WP + 127],
                A[1][0:128, c * WP: c * WP + 127]))
            cnt['dve'] += 1
            inc(i, 'sdve', 1)
            if c == 1:
                st['halo1'] = emit_halo_dual(1, ('sdve', a_mark),
                                             ('sdve', cnt['dve']))
            if c == 3:
                st['hmax_all'] = cnt['dve']

        def l0_chunk_evac(jj, c, pbase, pair, defer_hmax=False):
            # sign (ACT) -> vmax (DVE); hmax deferred when L1 of the previous
            # image is not yet emitted (prologue filler)
            st = S[jj]
            wait('act', 'spe', st['l0_pe'][c])
            if jj > 0 and 'vmax' in S[jj - 1]:
                wait('act', 'sdve', S[jj - 1]['vmax'][c])
            def mksgn(pbase=pbase, c=c):
                return nc.scalar.activation(
                    T0B[0:128, c * 508: c * 508 + 508],
                    PS[0:128, pbase: pbase + 508], AF.Sign)
            i = emit('act', mksgn)
            cnt['act'] += 1
            inc(i, 'sact', 1)
            pair_free[pair] = ('sact', cnt['act'])
            wait('dve', 'sact', cnt['act'])
            def mkv(c=c):
                a = T0B[0:128, c * 508: c * 508 + 254]
                b = T0B[0:128, c * 508 + 254: c * 508 + 508]
                return nc.vector.tensor_max(
                    T1B[0:128, c * 254: c * 254 + 254], a, b)
            i = emit('dve', mkv)
            cnt['dve'] += 1
            inc(i, 'sdve', 1)
            st.setdefault('vmax', {})[c] = cnt['dve']
            if not defer_hmax:
                emit_hmax(jj, c)

        def emit_l0_chunk_steady(jj, c, defer_hmax=False):
            # one L0 psum chunk (18 fp32r matmuls) + its evac chain
            st = S[jj]
            pair = take_pair()
            pbase = pair * 1024
            wait('pe', 'sdma', 32)      # W0F (both halves)
            marks = st['dma_marks']
            wait('pe', marks[-1][0], 16 * marks[-1][1])
            for t in range(2):
                for tap in range(9):
                    final = (t == 1 and tap == 8)
                    l0_mm(jj, c, t, tap, pbase, final)
                    if final:
                        st['l0_pe'][c] = cnt['pe']
            if c == 3:
                t_free[jj] = cnt['pe']
            l0_chunk_evac(jj, c, pbase, pair, defer_hmax=defer_hmax)

        def emit_l0_img0():
            # prologue: all 4 chunks of img 0, term-outer, 2 pairs
            st = S[0]
            st['l0_pe'] = [None] * 4
            p0, p1 = take_pair(), take_pair()
            st['c_pbase'] = [p0 * 1024, p0 * 1024 + 512,
                             p1 * 1024, p1 * 1024 + 512]
            st['c_pair'] = [p0, p0, p1, p1]
            wait('pe', 'sdma', 32)
            marks = st['dma_marks']
            # emission: chunk loop lives in emit_l0_chunk(0, 0); evacs per chunk
            for t in range(2):
                if t == 1:
                    wait('pe', marks[2][0], 16 * marks[2][1])
                else:
                    wait('pe', marks[0][0], 16 * marks[0][1])
                for cc in range(4):
                    if t == 0 and cc == 2:
                        wait('pe', marks[1][0], 16 * marks[1][1])
                    for tap in range(9):
                        final = (t == 1 and tap == 8)
                        l0_mm(0, cc, t, tap, st['c_pbase'][cc], final)
                        if final:
                            st['l0_pe'][cc] = cnt['pe']
            t_free[0] = cnt['pe']
            for cc in range(4):
                l0_chunk_evac(0, cc, st['c_pbase'][cc], st['c_pair'][cc])

        def dr_matmul(pbase, lhs_col, lhs_m, rhs_buf, rhs_off, rhs_delta,
                      n, start, stop, final):
            lstep = max(16, lhs_m)
            def mk():
                lv = WF8[0:128, lhs_col: lhs_col + lstep + lhs_m]
                lv = dataclasses.replace(lv, ap=[lv.ap[0], [lstep, 2], [1, lhs_m]])
                rv = rhs_buf[0:128, rhs_off: rhs_off + rhs_delta + n]
                rv = dataclasses.replace(rv, ap=[rv.ap[0], [rhs_delta, 2], [1, n]])
                ov = PS[0:lhs_m, pbase: pbase + n]
                return nc.tensor.matmul(ov, lv, rv, start=start, stop=stop,
                                        perf_mode=PM.DoubleRow)
            i = emit('pe', mk)
            if final:
                cnt['pe'] += 1
                inc(i, 'spe', 1)
            return i

        def pair_evac(eng, pair, nchunks, dstbuf, dstoff, pe_count, dst_ms,
                      sdma_guard, cb=None):
            # evacuate `nchunks` (1 or 2) 508-chunks of one pair -> contiguous;
            # when cb is set, also write the +2-shifted C copy at cb+2+dstoff
            wait(eng, 'spe', pe_count)
            if dst_ms is not None:
                wait(eng, 'sgp', dst_ms)
            if sdma_guard is not None:
                wait(eng, sdma_guard[0], sdma_guard[1])
            total = 508 * nchunks
            pbase = pair * 1024
            key = 'sact' if eng == 'act' else 'sdve'
            def mk(off):
                def f():
                    sv = PS[0:128, pbase: pbase + (nchunks - 1) * 512 + 508]
                    sv = dataclasses.replace(
                        sv, ap=[sv.ap[0], [512, nchunks], [1, 508]]) \
                        if nchunks > 1 else dataclasses.replace(
                            sv, ap=[sv.ap[0], [1, 508]])
                    dv = dstbuf[0:128, off: off + total]
                    if eng == 'act':
                        return nc.scalar.activation(dv, sv, AF.Sign)
                    return nc.vector.tensor_scalar(dv, sv, 1.0, -1.0,
                                                   ALU.min, ALU.max)
                return f
            i = emit(eng, mk(dstoff))
            cnt[eng] += 1
            inc(i, key, 1)
            a_mark = (key, cnt[eng])
            pair_free[pair] = a_mark
            if cb is not None:
                # C copy from the just-written A region (same engine, SBUF->
                # SBUF) -- cheaper than a psum re-read and frees the pair now
                def mkc():
                    sv2 = dstbuf[0:128, dstoff: dstoff + total]
                    dv2 = dstbuf[0:128, cb + 2 + dstoff: cb + 2 + dstoff + total]
                    if eng == 'act':
                        return nc.scalar.copy(dv2, sv2)
                    return nc.vector.tensor_copy(dv2, sv2)
                i = emit(eng, mkc)
                cnt[eng] += 1
                inc(i, key, 1)
            return a_mark, (key, cnt[eng])

        # halo geometry: (partition shift, first halo row) per layer
        HS = {1: (4, 4), 2: (8, 8), 3: (16, 16), 4: (32, 32), 6: (32, 32)}

        def emit_halo_dual(l, dep_a, dep_c):
            # partition-shifted band-boundary copy in A, then in C
            sh, rr = HS[l]
            def mk(off):
                def f():
                    return nc.sync.dma_start(
                        A[l][0:128 - sh, off + rr * WP: off + (rr + 2) * WP],
                        A[l][sh:128, off: off + 2 * WP])
                return f
            wait('sp', dep_a[0], dep_a[1])
            i = emit('sp', mk(0))
            hcnt[l] += 16
            inc(i, f'sh{l}', 16)
            wait('sp', dep_c[0], dep_c[1])
            i = emit('sp', mk(CBW[l] + 2))
            hcnt[l] += 16
            inc(i, f'sh{l}', 16)
            return (f'sh{l}', hcnt[l])

        # ---------------- per-tile emitters (layers 1..6) ----------------
        def pdelta(l, tA, tB):
            if tB is None:
                return 16
            if tB[0] == tA[0] + 2:
                return CBW[l] + 256   # cross pair (0,2)+(2,2) via the C copy
            return 128

        def emit_l1(j):
            st = S[j]
            pair = take_pair()
            pbase = pair * 1024
            wait('pe', 'sgp', MS_A[1])
            wait('pe', 'swf', 16)
            wait('pe', st['halo1'][0], st['halo1'][1])
            wait('pe', 'sdve', st['hmax_all'])
            for p, (tA, tB) in enumerate(PAIRS):
                for ph in range(2):
                    kiA, kjA = tA
                    dr_matmul(pbase + ph * 512, OFF[1] + (p * 2 + ph) * 256, 128,
                              A[1], kiA * WP + kjA, pdelta(1, tA, tB), 508,
                              start=(p == 0), stop=(p == 4),
                              final=(p == 4 and ph == 1))
            st['l1_done'] = cnt['pe']
            ev_a, ev = pair_evac('dve', pair, 2, A[2], 0, cnt['pe'], MS_A[2],
                                 ('sh2', hcnt[2]) if hcnt[2] else None,
                                 cb=CBW[2])
            st['l1_evac'] = ev
            st['halo2'] = emit_halo_dual(2, ev_a, ev)

        def emit_l2(j):
            st = S[j]
            prs = [take_pair(), take_pair()]
            wait('pe', 'sgp', MS_A[2])
            wait('pe', st['l1_evac'][0], st['l1_evac'][1])
            pe_marks = []
            for k in range(4):          # chunk k = (ph, jj): ph=k//2, jj=k%2
                ph, jj = k // 2, k % 2
                if jj == 1:
                    wait('pe', st['halo2'][0], st['halo2'][1])
                pb = prs[k // 2] * 1024 + (k % 2) * 512
                for p, (tA, tB) in enumerate(PAIRS):
                    kiA, kjA = tA
                    dr_matmul(pb, OFF[2] + (p * 2 + ph) * 256, 128,
                              A[2], (4 * jj + kiA) * WP + kjA,
                              pdelta(2, tA, tB), 508,
                              start=(p == 0), stop=(p == 4),
                              final=(p == 4 and k % 2 == 1))
                if k % 2 == 1:
                    pe_marks.append(cnt['pe'])
            ev0_a, ev0 = pair_evac('act', prs[0], 2, A[3], 0, pe_marks[0],
                                   MS_A[3],
                                   ('sh3', hcnt[3]) if hcnt[3] else None,
                                   cb=CBW[3])
            _, ev1 = pair_evac('dve', prs[1], 2, A[3], 8 * WP, pe_marks[1],
                               MS_A[3], None, cb=CBW[3])
            st['l2_evac_h1'] = ev0
            st['l2_evac'] = ev1
            st['halo3'] = emit_halo_dual(3, ev0_a, ev0)

        def emit_l3(j, tau):
            st = S[j]
            prs = [take_pair(), take_pair()]
            wait('pe', 'sgp', MS_A[3])
            wait('pe', st['l2_evac_h1'][0], st['l2_evac_h1'][1])
            pe_marks = []
            for jj in range(4):
                if jj == 1:
                    wait('pe', st['l2_evac'][0], st['l2_evac'][1])
                if jj == 3:
                    wait('pe', st['halo3'][0], st['halo3'][1])
                pb = prs[jj // 2] * 1024 + (jj % 2) * 512
                for p, (tA, tB) in enumerate(PAIRS):
                    kiA, kjA = tA
                    dr_matmul(pb, OFF[3] + (p * 2 + tau) * 256, 128,
                              A[3], (4 * jj + kiA) * WP + kjA,
                              pdelta(3, tA, tB), 508,
                              start=(p == 0), stop=(p == 4),
                              final=(p == 4 and jj % 2 == 1))
                if jj % 2 == 1:
                    pe_marks.append(cnt['pe'])
            evs = []
            for pi in range(2):
                evs.append(pair_evac(
                    'dve' if pi == 0 else 'act', prs[pi], 2, A[4],
                    tau * 16 * WP + pi * 1016,
                    pe_marks[pi], MS_A[4],
                    ('sh4', hcnt[4]) if (tau == 0 and pi == 0 and hcnt[4]) else None,
                    cb=CBW[4]))
            st.setdefault('l3_evacs', {})[tau] = [e[1] for e in evs]
            if tau == 0:
                st['halo4'] = emit_halo_dual(4, evs[0][0], evs[0][1])

        def emit_l4(j, h, tau):
            st = S[j]
            prs = [take_pair(), take_pair()]
            wait('pe', 'sgp', MS_A[4])
            for ev in st['l3_evacs'][tau]:
                wait('pe', ev[0], ev[1])
            pe_marks = []
            for jj in range(4):
                if jj == 3:
                    if tau == 0:
                        for ev in st['l3_evacs'][1]:
                            wait('pe', ev[0], ev[1])
                    else:
                        wait('pe', st['halo4'][0], st['halo4'][1])
                pb = prs[jj // 2] * 1024 + (jj % 2) * 512
                for p, (tA, tB) in enumerate(PAIRS):
                    kiA, kjA = tA
                    dr_matmul(pb, OFF[4] + (p * 2 + h) * 256, 128,
                              A[4], (16 * tau + 4 * jj + kiA) * WP + kjA,
                              pdelta(4, tA, tB), 508,
                              start=(p == 0), stop=(p == 4),
                              final=(p == 4 and jj % 2 == 1))
                if jj % 2 == 1:
                    pe_marks.append(cnt['pe'])
            evs = []
            for pi in range(2):
                _, f = pair_evac(
                    'act' if pi == 0 else 'dve', prs[pi], 2, A[5],
                    h * SL5 + tau * 16 * WP + pi * 1016, pe_marks[pi], MS_A[5],
                    ('sh5', hcnt[5]) if (h == 0 and tau == 0 and pi == 0
                                         and hcnt[5]) else None)
                evs.append(f)
            st.setdefault('l4_evacs', {})[(h, tau)] = evs
            if h == 1 and tau == 0:
                def mkh5():
                    sv = A[5][32:128, 0:SL5 + 2 * WP]
                    sv = dataclasses.replace(sv, ap=[sv.ap[0], [SL5, 2], [1, 2 * WP]])
                    dv = A[5][0:96, 32 * WP: SL5 + 34 * WP]
                    dv = dataclasses.replace(dv, ap=[dv.ap[0], [SL5, 2], [1, 2 * WP]])
                    return nc.sync.dma_start(dv, sv)
                wait('sp', st['l4_evacs'][(0, 0)][0][0],
                     st['l4_evacs'][(0, 0)][0][1])
                wait('sp', evs[0][0], evs[0][1])
                i = emit('sp', mkh5)
                hcnt[5] += 16
                inc(i, 'sh5', 16)
                st['halo5'] = ('sh5', hcnt[5])

        def emit_l5(j, tau):
            st = S[j]
            prs = [take_pair(), take_pair()]
            wait('pe', 'sgp', MS_A[5])
            if tau == 0:
                for ev in st['l4_evacs'][(1, 0)]:
                    wait('pe', ev[0], ev[1])
            for ev in st['l4_evacs'][(0, tau)] + st['l4_evacs'][(1, tau)]:
                wait('pe', ev[0], ev[1])
            pe_marks = []
            for jj in range(4):
                if jj == 3:
                    if tau == 0:
                        for ev in st['l4_evacs'][(1, 1)]:
                            wait('pe', ev[0], ev[1])
                    else:
                        wait('pe', st['halo5'][0], st['halo5'][1])
                pb = prs[jj // 2] * 1024 + (jj % 2) * 512
                for tap in range(9):
                    ki, kj = tap // 3, tap % 3
                    dr_matmul(pb, OFF[5] + tap * 256, 128,
                              A[5], (16 * tau + 4 * jj + ki) * WP + kj,
                              SL5, 508,
                              start=(tap == 0), stop=(tap == 8),
                              final=(tap == 8 and jj % 2 == 1))
                if jj % 2 == 1:
                    pe_marks.append(cnt['pe'])
            evs = []
            for pi in range(2):
                evs.append(pair_evac(
                    'dve' if pi == 0 else 'act', prs[pi], 2, A[6],
                    tau * 16 * WP + pi * 1016,
                    pe_marks[pi], MS_A[6],
                    ('sh6', hcnt[6]) if (tau == 0 and pi == 0 and hcnt[6]) else None,
                    cb=CBW[6]))
            st.setdefault('l5_evacs', {})[tau] = [e[1] for e in evs]
            if tau == 0:
                st['halo6'] = emit_halo_dual(6, evs[0][0], evs[0][1])

        def emit_l6(j, tau):
            st = S[j]
            prs = [take_pair(), take_pair()]
            wait('pe', 'sgp', MS_A[6])
            for ev in st['l5_evacs'][tau]:
                wait('pe', ev[0], ev[1])
            pe_marks = []
            for jj in range(4):
                if jj == 3:
                    if tau == 0:
                        for ev in st['l5_evacs'][1]:
                            wait('pe', ev[0], ev[1])
                    else:
                        wait('pe', st['halo6'][0], st['halo6'][1])
                pb = prs[jj // 2] * 1024 + (jj % 2) * 512
                for p, (tA, tB) in enumerate(PAIRS):
                    kiA, kjA = tA
                    dr_matmul(pb, OFF[6] + p * 32, 8,
                              A[6], (16 * tau + 4 * jj + kiA) * WP + kjA,
                              pdelta(6, tA, tB), 508,
                              start=(p == 0), stop=(p == 4),
                              final=(p == 4 and jj % 2 == 1))
                if jj % 2 == 1:
                    pe_marks.append(cnt['pe'])
            # evac per pair -> OUTB (fp8), then out-DMAs for this tau.
            # Last image: pair-1 evac runs on DVE (parallel with ACT) and the
            # out-DMAs split across Pool and SP to halve the final drain.
            last = (j == NIMG - 1)
            ev_marks = []
            for pi in range(2):
                eng = 'dve' if (last and pi == 1) else 'act'
                wait(eng, 'spe', pe_marks[pi])
                if tau == 0 and pi == 0 and out_cnt[0]:
                    wait('act', 'sout', out_cnt[0])
                def mk6(pi=pi, tau=tau, prs=prs, eng=eng):
                    pb = prs[pi] * 1024
                    sv = PS[0:8, pb: pb + 512 + 508]
                    sv = dataclasses.replace(
                        sv, ap=[sv.ap[0], [512, 2], [127, 4], [1, 115]])
                    dv = OUTB[0:8, tau * 16 * 115 + pi * 8 * 115:
                              tau * 16 * 115 + pi * 8 * 115 + 8 * 115]
                    if eng == 'act':
                        return nc.scalar.activation(dv, sv, AF.Sign)
                    return nc.vector.tensor_scalar(dv, sv, 1.0, -1.0,
                                                   ALU.min, ALU.max)
                i = emit(eng, mk6)
                key = 'sact' if eng == 'act' else 'sdve'
                cnt[eng] += 1
                inc(i, key, 1)
                pair_free[prs[pi]] = (key, cnt[eng])
                ev_marks.append((key, cnt[eng]))
            # out-DMAs for rows [16*tau, 16*tau+nr) -- off the SP halo queue
            k = 0
            for g in range(4):
                nr = 16 if (tau == 0 or g < 3) else 3
                for c in range(2):
                    qeng = 'sp' if (last and k % 2 == 1) else 'gp'
                    for mk in ev_marks:
                        wait(qeng, mk[0], mk[1])
                    def mko(j=j, g=g, c=c, nr=nr, tau=tau, qeng=qeng):
                        sv = OUTB[2 * g + c: 2 * g + c + 1,
                                  tau * 1840: tau * 1840 + nr * 115]
                        dv = y[j, 0:1]
                        dv = dataclasses.replace(
                            dv, offset=dv.offset + c * 13225
                            + (32 * g + 16 * tau) * 115,
                            ap=[[1, nr * 115]])
                        if qeng == 'sp':
                            return nc.sync.dma_start(dv, sv)
                        return nc.gpsimd.dma_start(dv, sv)
                    i = emit(qeng, mko)
                    if qeng == 'sp':
                        inc(i, 'sout2', 16)
                    else:
                        out_cnt[0] += 16
                        inc(i, 'sout', 16)
                    k += 1

        # ---------------- schedule ----------------
        emit_in_dma(0)
        emit_in_dma(1)
        for j in range(NIMG):
            S[j]['l0_pe'] = S[j].get('l0_pe', [None] * 4)
        emit_l0_img0()
        # prologue filler: img1's first L0 chunk covers img0's L0->L1 evac
        # latency (steady periods get this from the previous period's tail);
        # its hmax (A1 write) is deferred until after L1_0 is emitted
        S[1]['l0_pe'] = [None] * 4
        emit_l0_chunk_steady(1, 0, defer_hmax=True)
        for j in range(NIMG):
            if j + 2 < NIMG:
                emit_in_dma(j + 2)
            nxt = j + 1 if j + 1 < NIMG else None
            if nxt is not None and nxt > 1:
                S[nxt]['l0_pe'] = [None] * 4
            if j != 7:
                emit_l1(j)
            if j == 0:
                emit_hmax(1, 0)
                emit_l0_chunk_steady(1, 1)
            elif nxt is not None:
                emit_l0_chunk_steady(nxt, 0)
            if j != 7:
                emit_l2(j)
            if nxt is not None:
                emit_l0_chunk_steady(nxt, 2 if j == 0 else 1)
            if j != 7:
                emit_l3(j, 0)
            if nxt is not None:
                emit_l0_chunk_steady(nxt, 3 if j == 0 else 2)
            emit_l3(j, 1)
            emit_l4(j, 0, 0)
            emit_l4(j, 0, 1)
            if nxt is not None and j > 0:
                emit_l0_chunk_steady(nxt, 3)
            emit_l4(j, 1, 0)
            emit_l4(j, 1, 1)
            # img7's L1/L2 ride inside period 6 (like the L0 fillers) so the
            # filler-less last period starts directly at L3 with no stalls
            if j == 6:
                emit_l1(7)
            emit_l5(j, 0)
            if j == 6:
                emit_l2(7)
            emit_l5(j, 1)
            emit_l6(j, 0)
            if j == 6:
                emit_l3(7, 0)
            emit_l6(j, 1)
        return cnt

    with nc.Block() as block:
        @block.tensor
        def _(E):
            walk(E, 'pe')

        @block.scalar
        def _(E):
            walk(E, 'act')

        @block.vector
        def _(E):
            walk(E, 'dve')

        @block.gpsimd
        def _(E):
            walk(E, 'gp')

        @block.sync
        def _(E):
            walk(E, 'sp')

    for cm in reversed(ctxs):
        cm.__exit__(None, None, None)
    return nc


def round11(x):
    """Round fp32 array to 11-bit significand (RNE on low 13 mantissa bits)."""
    b = x.view(np.uint32).copy()
    low = b & np.uint32(0x1FFF)
    base = b & ~np.uint32(0x1FFF)
    rnd = (low > 0x1000) | ((low == 0x1000) & ((b >> 13) & 1).astype(bool))
    base = base + (rnd.astype(np.uint32) << 13)
    return base.view(np.float32)


def split_input(inp):
    """(N,3,256,256) fp32 -> (N,2,3,258,256): two 11-bit fp32r terms,
    rows zero-padded to 258 so band 31's halo loads in the main DMA."""
    t1 = round11(inp)
    t2 = round11((inp - t1).astype(np.float32))
    xs = np.zeros((inp.shape[0], 2, 3, 258, 256), np.float32)
    xs[:, 0, :, :256] = t1
    xs[:, 1, :, :256] = t2
    return xs


def pack_weights(ws):
    """ws: 7 raw arrays (cout, cin, 3, 3) -> (w0f fp32, wf8 fp8)."""
    import ml_dtypes
    sws = [np.sign(w).astype(np.float32) for w in ws]
    # L0: 32 bands x 3cin on 96 partitions -> 128 out (32 bands x 4)
    w0f = np.zeros((96, 9 * 128), np.float32)
    for tap in range(9):
        ki, kj = tap // 3, tap % 3
        blk = sws[0][:, :, ki, kj].T  # (cin, cout)
        for s in range(32):
            w0f[s * 3:s * 3 + 3, tap * 128 + s * 4: tap * 128 + s * 4 + 4] = blk
    wf8 = np.zeros((128, WF8_COLS), np.float32)
    # t2 layers: 1,2,3 (phases), 4 (cout halves), 6 (plain)
    for l, nph in ((1, 2), (2, 2), (3, 2)):
        cin, cout = CH[l]
        gin = G[l]
        M = 128
        for p, (tA, tB) in enumerate(PAIRS):
            for ph in range(nph):
                col = OFF[l] + (p * 2 + ph) * 256
                for i, tap in enumerate((tA, tB)):
                    if tap is None:
                        continue
                    ki, kj = tap
                    blk = sws[l][:, :, ki, kj].T  # (cin, cout)
                    for gp_ in range(gin // 2):
                        g = 2 * gp_ + ph
                        wf8[g * cin:(g + 1) * cin,
                            col + i * M + gp_ * cout: col + i * M + (gp_ + 1) * cout] = blk
    # L4: cout halves
    cin, cout = CH[4]
    for p, (tA, tB) in enumerate(PAIRS):
        for h in range(2):
            col = OFF[4] + (p * 2 + h) * 256
            for i, tap in enumerate((tA, tB)):
                if tap is None:
                    continue
                ki, kj = tap
                blk = sws[4][32 * h:32 * h + 32, :, ki, kj].T  # (32cin, 32cout)
                for g in range(4):
                    wf8[g * 32:(g + 1) * 32,
                        col + i * 128 + g * 32: col + i * 128 + (g + 1) * 32] = blk
    # L5: t1 k-split (slab i = channels 32i..32i+32)
    for tap in range(9):
        ki, kj = tap // 3, tap % 3
        col = OFF[5] + tap * 256
        for i in range(2):
            blk = sws[5][:, 32 * i:32 * i + 32, ki, kj].T  # (32cin-half, 32cout)
            for g in range(4):
                wf8[g * 32:(g + 1) * 32,
                    col + i * 128 + g * 32: col + i * 128 + (g + 1) * 32] = blk
    # L6: M=8 (ktile step padded to 16)
    for p, (tA, tB) in enumerate(PAIRS):
        col = OFF[6] + p * 32
        for i, tap in enumerate((tA, tB)):
            if tap is None:
                continue
            ki, kj = tap
            blk = sws[6][:, :, ki, kj].T  # (32, 2)
            for g in range(4):
                wf8[g * 32:(g + 1) * 32,
                    col + i * 16 + g * 2: col + i * 16 + (g + 1) * 2] = blk
    return w0f, wf8.astype(ml_dtypes.float8_e4m3fn)


LAST_RESULTS = None


def kernel(**inputs):
    global LAST_RESULTS
    from concourse.bass_utils import run_bass_kernel_spmd
    inp = np.asarray(inputs['inputs'], np.float32)
    ws = [np.asarray(inputs[f'w{i}']) for i in range(7)]
    w0f, wf8 = pack_weights(ws)
    nc = build_program()
    import ml_dtypes
    z8 = np.zeros((128, 9424), ml_dtypes.float8_e4m3fn)
    in_maps = []
    for c in range(8):
        xs = split_input(np.ascontiguousarray(inp[c * 8:(c + 1) * 8]))
        in_maps.append({'x': np.ascontiguousarray(xs),
                        'w0f': w0f, 'wf8': wf8, 'z8': z8})
    res = run_bass_kernel_spmd(nc, in_maps, core_ids=list(range(8)),
                               tmpdir=os.environ.get('KERNEL_TRACE_DIR') or None)
    LAST_RESULTS = res
    out = np.concatenate([np.asarray(res.results[c]['y'], np.float32)
                          for c in range(8)], axis=0)
    return out
